# revision 7
# baseline (speedup 1.0000x reference)
"""Trainium2 Bass kernel for nn_CausalSelfAttention_73358041415963.

Math (literal reference semantics):
  Q/K/V = per-head projections of X;  S = Q @ K^T (no scale, no mask)
  A = softmax(S, axis=QUERY)  -> each key-column normalized over queries
  AV = A @ V;  literal reshape (B,H,N,DV)->(B,N,H*DV);  out = AV_r @ W_O

Key structural facts exploited:
  * softmax over the query axis i means A = E / colsum(E) with E = exp(S);
    AV = E @ (V / n[:,None]) where n[jk] = sum_i E[i, jk] -- normalization
    folds into V rows, no pass over the big E matrix.
  * the literal reshape maps head h to output rows n' in [h*128,(h+1)*128),
    so head-sharding needs NO collectives: each core owns 2 heads = 256
    output rows per batch.
  * TimelineSim matmul cost = out-free-size * cycles_per_row only, so the
    AV product uses E^T blocks as the STATIONARY operand (lhsT) and scaled
    V as the 64-wide moving operand: half the PE rows of the avp-oriented
    version.  bf16 keeps 1.0 cycles/row at free<256 (f32r would be 4.0).

Sharding: 8 cores x 2 heads. Each core gets full X, its 2 heads' W_Q/W_K/W_V
(packed [D,128]), full W_O. Core c returns output rows [256c, 256c+256).
"""

import numpy as np

import concourse.tile as tile
from concourse import bacc, mybir
from concourse.bass_utils import run_bass_kernel_spmd
from concourse.masks import make_identity

F32 = mybir.dt.float32
BF16 = mybir.dt.bfloat16
P = 128
AF = mybir.ActivationFunctionType

# gpsimd software-DGE DMAs cast fp32 DRAM -> bf16 SBUF during the transfer.
CAST_DMA = True


def build_attn(tc, X, WQ, WK, WV, WO, O, N, D, DOUT):
    """Emit the per-core kernel into TileContext tc.

    X:  [2, N, D]    (full input, fp32)
    WQ/WK/WV: [D, 128]   2 local heads packed along the last axis
    WO: [16*64, DOUT]
    O:  [2, 2*(N//16), DOUT]   output rows for the 2 local heads
    """
    nc = tc.nc
    B, HL, SG = 2, 2, 16
    DCH = D // 128        # contraction chunks over model dim
    NCH = N // 512        # 512-wide chunks of sequence
    JKB = N // 128        # key blocks
    IHALF = N // 2        # scores processed in two i-halves
    CS = 512
    NCPH = IHALF // CS
    R = N // 16           # output rows per head

    with (
        tc.tile_pool(name="persist", bufs=1) as pp,
    ):
        ident = pp.tile([P, P], F32, tag="ident", name="ident")
        make_identity(nc, ident)
        identb = pp.tile([P, P], BF16, tag="identb", name="identb")
        nc.vector.tensor_copy(identb, ident)
        # Dummy exp: forces the ACT Exp table load during the prologue
        # instead of at the first real score-exp.
        warm = pp.tile([P, 1], F32, tag="warm", name="warm")
        nc.scalar.activation(warm, ident[:, 0:1], AF.Exp)
        # zero rhs for the explicit PSUM-bank zeroing matmuls
        zb = pp.tile([P, 512], BF16, tag="zb", name="zb")
        nc.vector.memset(zb, 0.0)

        wq_sb = pp.tile([P, DCH, P], BF16, tag="wq", name="wq_sb")
        wk_sb = pp.tile([P, DCH, P], BF16, tag="wk", name="wk_sb")
        wv_sb = pp.tile([P, DCH, P], BF16, tag="wv", name="wv_sb")
        if CAST_DMA:
            nc.gpsimd.dma_start(wq_sb, WQ.rearrange("(dc p) m -> p dc m", p=P))
            nc.gpsimd.dma_start(wk_sb, WK.rearrange("(dc p) m -> p dc m", p=P))
            nc.gpsimd.dma_start(wv_sb, WV.rearrange("(dc p) m -> p dc m", p=P))
        else:
            wq_f = pp.tile([P, DCH, P], F32, tag="wqf", name="wq_f")
            wk_f = pp.tile([P, DCH, P], F32, tag="wkf", name="wk_f")
            wv_f = pp.tile([P, DCH, P], F32, tag="wvf", name="wv_f")
            nc.sync.dma_start(wq_f, WQ.rearrange("(dc p) m -> p dc m", p=P))
            nc.sync.dma_start(wk_f, WK.rearrange("(dc p) m -> p dc m", p=P))
            nc.sync.dma_start(wv_f, WV.rearrange("(dc p) m -> p dc m", p=P))
            nc.gpsimd.tensor_copy(wq_sb, wq_f)
            nc.gpsimd.tensor_copy(wk_sb, wk_f)
            nc.gpsimd.tensor_copy(wv_sb, wv_f)

        qT, kT, v_sb, avr = [], [], [], []
        for b in range(B):
            qT.append(pp.tile([P, N], BF16, tag=f"qT{b}", name=f"qT{b}"))
            kT.append(pp.tile([P, N], BF16, tag=f"kT{b}", name=f"kT{b}"))
            v_sb.append(pp.tile([P, JKB, P], BF16, tag=f"v{b}", name=f"v{b}"))
            avr.append(pp.tile([P, N], BF16, tag=f"avr{b}", name=f"avr{b}"))

        # ---------------- Phase P: X^T, projections ----------------
        with (
            tc.tile_pool(name="pP", bufs=1) as sp,
            tc.tile_pool(name="pPps", bufs=1, space="PSUM") as pps,
        ):
            def emit_vtrans(pend):
                vt_pend, b_pend, nch_pend = pend
                tp2 = pps.tile([P, 512], BF16, tag="tp", bufs=4, name="tp2")
                for ns in range(4):
                    nc.tensor.transpose(
                        tp2[:, ns * 128 : (ns + 1) * 128],
                        vt_pend[:, ns * 128 : (ns + 1) * 128],
                        identb,
                    )
                nc.vector.tensor_copy(
                    v_sb[b_pend][:, nch_pend * 4 : (nch_pend + 1) * 4, :], tp2
                )

            pending_vt = None
            for b in range(B):
                for nch in range(NCH):
                    xns = []
                    for ns in range(4):
                        xn = sp.tile([P, D], BF16, tag="xn", bufs=8, name="xn")
                        n0 = nch * 512 + ns * 128
                        if CAST_DMA:
                            nc.gpsimd.dma_start(xn, X[b, n0 : n0 + 128, :])
                        else:
                            xf = sp.tile([P, D], F32, tag="xf", bufs=4, name="xf")
                            nc.sync.dma_start(xf, X[b, n0 : n0 + 128, :])
                            nc.gpsimd.tensor_copy(xn, xf)
                        xns.append(xn)
                    if pending_vt is not None:
                        emit_vtrans(pending_vt)
                        pending_vt = None
                    # Q^T, K^T, V^T for this n-chunk (2 heads packed on
                    # partitions). Per d-chunk: transpose X block, copy to
                    # SBUF, immediately run the 3 accumulating projections.
                    qt_ps = pps.tile([P, 512], F32, tag="qk", bufs=4, name="qt_ps")
                    kt_ps = pps.tile([P, 512], F32, tag="qk", bufs=4, name="kt_ps")
                    vt_ps = pps.tile([P, 512], F32, tag="qk", bufs=4, name="vt_ps")
                    # Software-pipelined by one d-chunk: the PE queue is
                    # in-order, so the projection matmuls for chunk dc are
                    # emitted after chunk dc+1's transposes — the PE streams
                    # transposes while the copy for dc drains on DVE.
                    def emit_mms(dc, xtc):
                        nc.tensor.matmul(
                            qt_ps, wq_sb[:, dc, :], xtc,
                            start=(dc == 0), stop=(dc == DCH - 1),
                        )
                        nc.tensor.matmul(
                            kt_ps, wk_sb[:, dc, :], xtc,
                            start=(dc == 0), stop=(dc == DCH - 1),
                        )
                        nc.tensor.matmul(
                            vt_ps, wv_sb[:, dc, :], xtc,
                            start=(dc == 0), stop=(dc == DCH - 1),
                        )

                    prev = None
                    for dc in range(DCH):
                        tp = pps.tile([P, 512], BF16, tag="tp", bufs=4, name="tp")
                        for ns in range(4):
                            nc.tensor.transpose(
                                tp[:, ns * 128 : (ns + 1) * 128],
                                xns[ns][:, dc * 128 : (dc + 1) * 128],
                                identb,
                            )
                        xtc = sp.tile([P, 512], BF16, tag="xtc", bufs=6, name="xtc")
                        nc.vector.tensor_copy(xtc, tp)
                        if prev is not None:
                            emit_mms(dc - 1, prev)
                        prev = xtc
                    emit_mms(DCH - 1, prev)
                    nc.vector.tensor_copy(qT[b][:, nch * 512 : (nch + 1) * 512], qt_ps)
                    nc.vector.tensor_copy(kT[b][:, nch * 512 : (nch + 1) * 512], kt_ps)
                    # V^T -> V natural via PE transposes, deferred to the
                    # start of the next chunk so the vt_sb drain never
                    # stalls the in-order PE queue.
                    vt_sb = sp.tile([P, 512], BF16, tag="vt", bufs=3, name="vt_sb")
                    nc.vector.tensor_copy(vt_sb, vt_ps)
                    pending_vt = (vt_sb, b, nch)

            if pending_vt is not None:
                emit_vtrans(pending_vt)
                pending_vt = None

        # Pre-issue the replicated W_O loads (no upstream deps -> their DMAs
        # overlap the projection/attention phases; bufs=2 pipelines the rest).
        OC = 256  # output column chunk
        wo_src = WO.rearrange("(s dv) d -> dv s d", dv=64)
        wo_tiles = []
        for dh in range(DOUT // OC):
            wo_t = pp.tile([P, SG, OC], BF16, tag="wo", bufs=4, name="wo_t")
            if CAST_DMA:
                # split over s to keep each software-DGE under 256 descriptors
                for s4 in range(4):
                    nc.gpsimd.dma_start(
                        wo_t[0:64, s4 * 4 : (s4 + 1) * 4, :],
                        wo_src[:, s4 * 4 : (s4 + 1) * 4, dh * OC : (dh + 1) * OC],
                    )
            else:
                wo_f = pp.tile([64, SG, OC], F32, tag="wof", bufs=2, name="wo_f")
                nc.sync.dma_start(wo_f, wo_src[:, :, dh * OC : (dh + 1) * OC])
                nc.gpsimd.tensor_copy(wo_t[0:64], wo_f)
            nc.sync.dma_start(wo_t[64:128], wo_t[0:64])
            wo_tiles.append(wo_t)

        # ---------------- Phase A: scores, exp, AV ----------------
        with (
            tc.tile_pool(name="pA", bufs=1) as ab,
            tc.tile_pool(name="pAps", bufs=1, space="PSUM") as aps,
        ):
            def emit_wo_chunk(wb, idx):
                # Output projection chunk (one (dh, head) pair) for batch
                # wb; op tiles borrow "st"-tag PSUM slots.
                dh, h = idx // HL, idx % HL
                wo_t = wo_tiles[dh]
                avv = avr[wb].rearrange("p (r s) -> p s r", s=SG)
                hs = slice(h * 64, (h + 1) * 64)
                opf = aps.tile([P, IHALF], F32, tag="st", bufs=2, name="opf")
                op = opf[:R, :OC]
                for s in range(SG):
                    nc.tensor.matmul(
                        op,
                        avv[hs, s, :],
                        wo_t[hs, s, :],
                        start=(s == 0), stop=(s == SG - 1),
                    )
                o_t = pp.tile([R, OC], F32, tag="ot", bufs=2, name="o_t")
                nc.vector.tensor_copy(o_t, op)
                nc.sync.dma_start(
                    O[wb, h * R : (h + 1) * R, dh * OC : (dh + 1) * OC], o_t
                )

            wo_pending = []
            for b in range(B):
                # AV^T accumulators: [queries-of-iblock, 64 dv] tiles packed
                # as avt[h][:, ib*64:(ib+1)*64], accumulated over all jk.
                avt = []
                for h in range(HL):
                    a = aps.tile([P, JKB * 64], F32, tag=f"avt{h}", bufs=1,
                                 name=f"avt{h}")
                    avt.append(a)
                    # Explicitly zero the banks with full-bank matmuls so the
                    # later 64-col accumulating writes (start=False) always
                    # land on zeroed PSUM regardless of lazy-zero semantics.
                    for bank in range(JKB * 64 // 512):
                        nc.tensor.matmul(
                            a[:, bank * 512 : (bank + 1) * 512],
                            identb, zb, start=True, stop=False,
                            skip_group_check=True,
                        )
                for jk in range(JKB):
                    # interleave pending W_O chunks (previous batch) between
                    # jk iterations so they hide under this batch's exp time
                    # instead of stalling the ACT pipeline or trailing.
                    if wo_pending and jk % 2 == 0:
                        emit_wo_chunk(*wo_pending.pop(0))
                    es = {}
                    nsum = {}
                    for h in range(HL):
                        nsum[h] = ab.tile([P, 2], F32, tag="nsum", bufs=4, name="nsum")
                    for half in range(2):
                        st = {}
                        for h in range(HL):
                            st[h] = aps.tile(
                                [P, IHALF], F32, tag="st", bufs=2, name="st"
                            )
                        for c in range(NCPH):
                            for h in range(HL):
                                hs = slice(h * 64, (h + 1) * 64)
                                i0 = half * IHALF + c * CS
                                nc.tensor.matmul(
                                    st[h][:, c * CS : (c + 1) * CS],
                                    kT[b][hs, jk * 128 : (jk + 1) * 128],
                                    qT[b][hs, i0 : i0 + CS],
                                    start=True, stop=True,
                                )
                        for h in range(HL):
                            e = ab.tile([P, IHALF], BF16, tag="e", bufs=8, name="e")
                            nc.scalar.activation(
                                e, st[h], AF.Exp,
                                accum_out=nsum[h][:, half : half + 1],
                            )
                            es[(h, half)] = e
                    vsp = {}
                    for h in range(HL):
                        hs = slice(h * 64, (h + 1) * 64)
                        n1 = ab.tile([P, 1], F32, tag="n1", bufs=4, name="n1")
                        nc.vector.reduce_sum(n1, nsum[h], axis=mybir.AxisListType.X)
                        nr = ab.tile([P, 1], F32, tag="nr", bufs=4, name="nr")
                        nc.vector.reciprocal(nr, n1)
                        vs = ab.tile([P, 64], BF16, tag="vs", bufs=6, name="vs")
                        nc.vector.tensor_scalar_mul(vs, v_sb[b][:, jk, hs], nr)
                        vsp[h] = vs
                    # AV^T: es block is the stationary lhsT (free dim 128 ->
                    # out partitions = queries), scaled V the 64-wide moving
                    # operand.  Accumulates over jk into pre-zeroed banks.
                    for h in range(HL):
                        for half in range(2):
                            for c in range(NCPH):
                                for ib2 in range(CS // 128):
                                    ib = half * 8 + c * 4 + ib2
                                    nc.tensor.matmul(
                                        avt[h][:, ib * 64 : (ib + 1) * 64],
                                        es[(h, half)][
                                            :, (c * 4 + ib2) * 128
                                            : (c * 4 + ib2 + 1) * 128
                                        ],
                                        vsp[h],
                                        start=False,
                                        stop=(jk == JKB - 1),
                                        skip_group_check=True,
                                    )
                # Drain: avt (PSUM fp32, [q, dv] layout) -> bf16 SBUF ->
                # PE-transpose back to [dv, q] -> avr rows for W_O.
                avts = {}
                for h in range(HL):
                    a_sb = ab.tile([P, JKB * 64], BF16, tag="avts", bufs=2,
                                   name="avts")
                    nc.vector.tensor_copy(a_sb, avt[h])
                    avts[h] = a_sb
                for h in range(HL):
                    ps = aps.tile([P, N], BF16, tag="st", bufs=2, name="ps")
                    for ib in range(JKB):
                        nc.tensor.transpose(
                            ps[0:64, ib * 128 : (ib + 1) * 128],
                            avts[h][:, ib * 64 : (ib + 1) * 64],
                            identb,
                        )
                    nc.vector.tensor_copy(avr[b][h * 64 : (h + 1) * 64, :],
                                          ps[0:64, :])
                wo_pending.extend((b, idx) for idx in range((DOUT // OC) * HL))
            for args in wo_pending:
                emit_wo_chunk(*args)


def build_nc(N=2048, D=1024, DOUT=1024, enable_asserts=False):
    """Build and compile the per-core Bass module. Returns nc."""
    nc = bacc.Bacc(
        "TRN2",
        target_bir_lowering=False,
        debug=False,
        enable_asserts=enable_asserts,
    )
    R = N // 16
    X = nc.dram_tensor("X", [2, N, D], F32, kind="ExternalInput").ap()
    WQ = nc.dram_tensor("WQ", [D, 128], F32, kind="ExternalInput").ap()
    WK = nc.dram_tensor("WK", [D, 128], F32, kind="ExternalInput").ap()
    WV = nc.dram_tensor("WV", [D, 128], F32, kind="ExternalInput").ap()
    WO = nc.dram_tensor("WO", [16 * 64, DOUT], F32, kind="ExternalInput").ap()
    O = nc.dram_tensor("O", [2, 2 * R, DOUT], F32, kind="ExternalOutput").ap()
    with tile.TileContext(nc) as tc:
        build_attn(tc, X, WQ, WK, WV, WO, O, N, D, DOUT)
    nc.compile()
    return nc


_NC_CACHE = {}


def _get_nc():
    if "full" not in _NC_CACHE:
        _NC_CACHE["full"] = build_nc()
    return _NC_CACHE["full"]


class _PjrtRunner:
    """Cached jitted SPMD executor (mirrors bass2jax.run_bass_via_pjrt but
    keeps the jitted callable so repeat calls skip re-trace/re-compile)."""

    def __init__(self, nc, n_cores=8):
        import jax
        from jax.experimental.shard_map import shard_map
        from jax.sharding import Mesh, PartitionSpec
        from concourse import bass2jax

        bass2jax.install_neuronx_cc_hook()
        self.n_cores = n_cores
        partition_name = (
            nc.partition_id_tensor.name if nc.partition_id_tensor else None
        )
        in_names, out_names, out_avals, zero_outs = [], [], [], []
        for alloc in nc.m.functions[0].allocations:
            if not isinstance(alloc, mybir.MemoryLocationSet):
                continue
            name = alloc.memorylocations[0].name
            if alloc.kind == "ExternalInput":
                if name != partition_name:
                    in_names.append(name)
            elif alloc.kind == "ExternalOutput":
                out_names.append(name)
                shape = tuple(alloc.tensor_shape)
                dtype = mybir.dt.np(alloc.dtype)
                out_avals.append(jax.core.ShapedArray(shape, dtype))
                zero_outs.append(np.zeros(shape, dtype))
        self.in_names = in_names
        self.out_names = out_names
        self.out_avals = out_avals
        self.zero_outs = zero_outs
        n_params = len(in_names)
        n_outs = len(out_names)
        all_names = list(in_names + out_names)
        if partition_name is not None:
            all_names.append(partition_name)
        all_names = tuple(all_names)

        def _body(*args):
            operands = list(args)
            if partition_name is not None:
                operands.append(bass2jax.partition_id_tensor())
            outs = bass2jax._bass_exec_p.bind(
                *operands,
                out_avals=tuple(out_avals),
                in_names=all_names,
                out_names=tuple(out_names),
                lowering_input_output_aliases=(),
                sim_require_finite=True,
                sim_require_nnan=True,
                nc=nc,
            )
            return tuple(outs)

        devices = jax.devices()[:n_cores]
        mesh = Mesh(np.asarray(devices), ("core",))
        donate = tuple(range(n_params, n_params + n_outs))
        self._fn = jax.jit(
            shard_map(
                _body,
                mesh=mesh,
                in_specs=(PartitionSpec("core"),) * (n_params + n_outs),
                out_specs=(PartitionSpec("core"),) * n_outs,
                check_rep=False,
            ),
            donate_argnums=donate,
            keep_unused=True,
        )

    def __call__(self, in_maps):
        import jax

        n = self.n_cores
        concat_in = [
            np.concatenate([np.asarray(m[nm]) for m in in_maps], axis=0)
            for nm in self.in_names
        ]
        concat_zeros = [
            np.zeros((n * z.shape[0], *z.shape[1:]), z.dtype) for z in self.zero_outs
        ]
        outs = self._fn(*concat_in, *concat_zeros)
        outs = [np.asarray(o) for o in jax.block_until_ready(outs)]
        return [
            {
                nm: outs[i].reshape(n, *self.out_avals[i].shape)[c]
                for i, nm in enumerate(self.out_names)
            }
            for c in range(n)
        ]


def _get_runner():
    if "runner" not in _NC_CACHE:
        _NC_CACHE["runner"] = _PjrtRunner(_get_nc())
    return _NC_CACHE["runner"]


def _make_in_maps(X, W_Q, W_K, W_V, W_O):
    X = np.ascontiguousarray(np.asarray(X), dtype=np.float32)
    W_Q = np.asarray(W_Q, dtype=np.float32)
    W_K = np.asarray(W_K, dtype=np.float32)
    W_V = np.asarray(W_V, dtype=np.float32)
    W_O = np.ascontiguousarray(np.asarray(W_O), dtype=np.float32)
    in_maps = []
    for c in range(8):
        wq = np.ascontiguousarray(
            np.concatenate([W_Q[2 * c], W_Q[2 * c + 1]], axis=1), dtype=np.float32
        )
        wk = np.ascontiguousarray(
            np.concatenate([W_K[2 * c], W_K[2 * c + 1]], axis=1), dtype=np.float32
        )
        wv = np.ascontiguousarray(
            np.concatenate([W_V[2 * c], W_V[2 * c + 1]], axis=1), dtype=np.float32
        )
        in_maps.append({"X": X, "WQ": wq, "WK": wk, "WV": wv, "WO": W_O})
    return in_maps


def kernel_with_results(X, W_Q, W_K, W_V, W_O, **run_kwargs):
    """Run via run_bass_kernel_spmd (supports trace kwargs); returns results."""
    nc = _get_nc()
    in_maps = _make_in_maps(X, W_Q, W_K, W_V, W_O)
    res = run_bass_kernel_spmd(nc, in_maps, core_ids=list(range(8)), **run_kwargs)
    return np.concatenate([r["O"] for r in res.results], axis=1), res


def kernel(X, W_Q, W_K, W_V, W_O):
    """Full-input entry point. X [2,2048,1024], W_Q/K/V [16,1024,64],
    W_O [1024,1024] -> [2,2048,1024] fp32."""
    try:
        runner = _get_runner()
        results = runner(_make_in_maps(X, W_Q, W_K, W_V, W_O))
        return np.concatenate([r["O"] for r in results], axis=1)
    except Exception:
        out, _ = kernel_with_results(X, W_Q, W_K, W_V, W_O)
        return out


# revision 14
# speedup vs baseline: 1.0296x; 1.0296x over previous
"""Trainium2 Bass kernel for nn_CausalSelfAttention_73358041415963.

Math (literal reference semantics):
  Q/K/V = per-head projections of X;  S = Q @ K^T (no scale, no mask)
  A = softmax(S, axis=QUERY)  -> each key-column normalized over queries
  AV = A @ V;  literal reshape (B,H,N,DV)->(B,N,H*DV);  out = AV_r @ W_O

Key structural facts exploited:
  * softmax over the query axis i means A = E / colsum(E) with E = exp(S);
    AV = E @ (V / n[:,None]) where n[jk] = sum_i E[i, jk] -- normalization
    folds into V rows, no pass over the big E matrix.
  * the literal reshape maps head h to output rows n' in [h*128,(h+1)*128),
    so head-sharding needs NO collectives: each core owns 2 heads = 256
    output rows per batch.
  * TimelineSim matmul cost = out-free-size * cycles_per_row only, so the
    AV product uses E^T blocks as the STATIONARY operand (lhsT) and scaled
    V as the 64-wide moving operand: half the PE rows of the avp-oriented
    version.  bf16 keeps 1.0 cycles/row at free<256 (f32r would be 4.0).

Sharding: 8 cores x 2 heads. Each core gets full X, its 2 heads' W_Q/W_K/W_V
(packed [D,128]), full W_O. Core c returns output rows [256c, 256c+256).
"""

import numpy as np

import concourse.tile as tile
from concourse import bacc, mybir
from concourse.bass_utils import run_bass_kernel_spmd
from concourse.masks import make_identity

F32 = mybir.dt.float32
BF16 = mybir.dt.bfloat16
P = 128
AF = mybir.ActivationFunctionType

# gpsimd software-DGE DMAs cast fp32 DRAM -> bf16 SBUF during the transfer.
CAST_DMA = True


def build_attn(tc, X, WQ, WK, WV, WO, O, N, D, DOUT):
    """Emit the per-core kernel into TileContext tc.

    X:  [2, N, D]    (full input, fp32)
    WQ/WK/WV: [D, 128]   2 local heads packed along the last axis
    WO: [16*64, DOUT]
    O:  [2, 2*(N//16), DOUT]   output rows for the 2 local heads
    """
    nc = tc.nc
    B, HL, SG = 2, 2, 16
    DCH = D // 128        # contraction chunks over model dim
    NCH = N // 512        # 512-wide chunks of sequence
    JKB = N // 128        # key blocks
    IHALF = N // 2        # scores processed in two i-halves
    CS = 512
    NCPH = IHALF // CS
    R = N // 16           # output rows per head

    with (
        tc.tile_pool(name="persist", bufs=1) as pp,
    ):
        ident = pp.tile([P, P], F32, tag="ident", name="ident")
        make_identity(nc, ident)
        identb = pp.tile([P, P], BF16, tag="identb", name="identb")
        nc.vector.tensor_copy(identb, ident)
        # Dummy exp: forces the ACT Exp table load during the prologue
        # instead of at the first real score-exp.
        warm = pp.tile([P, 1], F32, tag="warm", name="warm")
        nc.scalar.activation(warm, ident[:, 0:1], AF.Exp)
        # zero rhs for the explicit PSUM-bank zeroing matmuls
        zb = pp.tile([P, 512], BF16, tag="zb", name="zb")
        nc.vector.memset(zb, 0.0)

        wq_sb = pp.tile([P, DCH, P], BF16, tag="wq", name="wq_sb")
        wk_sb = pp.tile([P, DCH, P], BF16, tag="wk", name="wk_sb")
        wv_sb = pp.tile([P, DCH, P], BF16, tag="wv", name="wv_sb")
        if CAST_DMA:
            nc.gpsimd.dma_start(wq_sb, WQ.rearrange("(dc p) m -> p dc m", p=P))
            nc.gpsimd.dma_start(wk_sb, WK.rearrange("(dc p) m -> p dc m", p=P))
            nc.gpsimd.dma_start(wv_sb, WV.rearrange("(dc p) m -> p dc m", p=P))
        else:
            wq_f = pp.tile([P, DCH, P], F32, tag="wqf", name="wq_f")
            wk_f = pp.tile([P, DCH, P], F32, tag="wkf", name="wk_f")
            wv_f = pp.tile([P, DCH, P], F32, tag="wvf", name="wv_f")
            nc.sync.dma_start(wq_f, WQ.rearrange("(dc p) m -> p dc m", p=P))
            nc.sync.dma_start(wk_f, WK.rearrange("(dc p) m -> p dc m", p=P))
            nc.sync.dma_start(wv_f, WV.rearrange("(dc p) m -> p dc m", p=P))
            nc.gpsimd.tensor_copy(wq_sb, wq_f)
            nc.gpsimd.tensor_copy(wk_sb, wk_f)
            nc.gpsimd.tensor_copy(wv_sb, wv_f)

        qT, kT, v_sb = [], [], []
        for b in range(B):
            qT.append(pp.tile([P, N], BF16, tag=f"qT{b}", name=f"qT{b}"))
            kT.append(pp.tile([P, N], BF16, tag=f"kT{b}", name=f"kT{b}"))
            v_sb.append(pp.tile([P, JKB, P], BF16, tag=f"v{b}", name=f"v{b}"))

        # Prefetch ALL of X up front as 8 wide cast-DMAs (one per 512-row
        # chunk): far fewer software-DGE descriptor-gen rounds on Pool, and
        # the transfers pipeline ahead of the transpose consumer.
        xn4s = {}
        for b in range(B):
            for nch in range(NCH):
                xn4 = pp.tile([P, 4, D], BF16, tag="xn4", bufs=8, name="xn4")
                n0 = nch * 512
                nc.gpsimd.dma_start(
                    xn4, X[b, n0 : n0 + 512, :].rearrange("(ns p) d -> p ns d", p=P)
                )
                xn4s[(b, nch)] = xn4

        # ---------------- Phase P: X^T, projections ----------------
        with (
            tc.tile_pool(name="pP", bufs=1) as sp,
            tc.tile_pool(name="pPps", bufs=1, space="PSUM") as pps,
        ):
            def emit_vtrans(pend):
                vt_pend, b_pend, nch_pend = pend
                tp2 = pps.tile([P, 512], BF16, tag="tp", bufs=4, name="tp2")
                for ns in range(4):
                    nc.tensor.transpose(
                        tp2[:, ns * 128 : (ns + 1) * 128],
                        vt_pend[:, ns * 128 : (ns + 1) * 128],
                        identb,
                    )
                nc.vector.tensor_copy(
                    v_sb[b_pend][:, nch_pend * 4 : (nch_pend + 1) * 4, :], tp2
                )

            pending_vt = None
            for b in range(B):
                for nch in range(NCH):
                    xn4 = xn4s[(b, nch)]
                    xns = [xn4[:, ns, :] for ns in range(4)]
                    if pending_vt is not None:
                        emit_vtrans(pending_vt)
                        pending_vt = None
                    # Q^T, K^T, V^T for this n-chunk (2 heads packed on
                    # partitions). Per d-chunk: transpose X block, copy to
                    # SBUF, immediately run the 3 accumulating projections.
                    qt_ps = pps.tile([P, 512], F32, tag="qk", bufs=4, name="qt_ps")
                    kt_ps = pps.tile([P, 512], F32, tag="qk", bufs=4, name="kt_ps")
                    vt_ps = pps.tile([P, 512], F32, tag="qk", bufs=4, name="vt_ps")
                    # Software-pipelined by one d-chunk: the PE queue is
                    # in-order, so the projection matmuls for chunk dc are
                    # emitted after chunk dc+1's transposes — the PE streams
                    # transposes while the copy for dc drains on DVE.
                    def emit_mms(dc, xtc):
                        nc.tensor.matmul(
                            qt_ps, wq_sb[:, dc, :], xtc,
                            start=(dc == 0), stop=(dc == DCH - 1),
                        )
                        nc.tensor.matmul(
                            kt_ps, wk_sb[:, dc, :], xtc,
                            start=(dc == 0), stop=(dc == DCH - 1),
                        )
                        nc.tensor.matmul(
                            vt_ps, wv_sb[:, dc, :], xtc,
                            start=(dc == 0), stop=(dc == DCH - 1),
                        )

                    prev = None
                    for dc in range(DCH):
                        tp = pps.tile([P, 512], BF16, tag="tp", bufs=4, name="tp")
                        for ns in range(4):
                            nc.tensor.transpose(
                                tp[:, ns * 128 : (ns + 1) * 128],
                                xns[ns][:, dc * 128 : (dc + 1) * 128],
                                identb,
                            )
                        xtc = sp.tile([P, 512], BF16, tag="xtc", bufs=6, name="xtc")
                        nc.vector.tensor_copy(xtc, tp)
                        if prev is not None:
                            emit_mms(dc - 1, prev)
                        prev = xtc
                    emit_mms(DCH - 1, prev)
                    nc.vector.tensor_copy(qT[b][:, nch * 512 : (nch + 1) * 512], qt_ps)
                    nc.vector.tensor_copy(kT[b][:, nch * 512 : (nch + 1) * 512], kt_ps)
                    # V^T -> V natural via PE transposes, deferred to the
                    # start of the next chunk so the vt_sb drain never
                    # stalls the in-order PE queue.
                    vt_sb = sp.tile([P, 512], BF16, tag="vt", bufs=3, name="vt_sb")
                    nc.vector.tensor_copy(vt_sb, vt_ps)
                    pending_vt = (vt_sb, b, nch)

            if pending_vt is not None:
                emit_vtrans(pending_vt)
                pending_vt = None

        # W_O in natural row layout: chunk ic holds rows i in
        # [128*ic, 128*(ic+1)) on partitions — the W_O matmul contracts the
        # full 128-wide (s,dv) chunk at once (half the PE rows of the
        # 64-contraction variant) and needs no partition broadcast.
        OC = 256  # output column chunk
        wo2 = []
        for ic in range(DOUT // P):
            wo_t = pp.tile([P, DOUT], BF16, tag="wo", bufs=8, name="wo_t")
            nc.gpsimd.dma_start(wo_t, WO[ic * P : (ic + 1) * P, :])
            wo2.append(wo_t)

        # ---------------- Phase A: scores, exp, AV ----------------
        with (
            tc.tile_pool(name="pA", bufs=1) as ab,
            tc.tile_pool(name="pAps", bufs=1, space="PSUM") as aps,
        ):
            avct = {}

            def emit_wo_chunk(wb, idx):
                # Output projection chunk (one (dh, head) pair) for batch
                # wb; op tiles borrow "st"-tag PSUM slots.  Contracts the
                # full 128-wide (s,dv) chunks of AVc^T against natural-row
                # W_O chunks.
                dh, h = idx // HL, idx % HL
                act = avct[(wb, h)]
                opf = aps.tile([P, IHALF], F32, tag="st", bufs=2, name="opf")
                op = opf[:R, :OC]
                for ic in range(DOUT // P):
                    nc.tensor.matmul(
                        op,
                        act[:, ic, :],
                        wo2[ic][:, dh * OC : (dh + 1) * OC],
                        start=(ic == 0), stop=(ic == DOUT // P - 1),
                    )
                o_t = pp.tile([R, OC], F32, tag="ot", bufs=2, name="o_t")
                nc.vector.tensor_copy(o_t, op)
                nc.sync.dma_start(
                    O[wb, h * R : (h + 1) * R, dh * OC : (dh + 1) * OC], o_t
                )

            wo_pending = []
            for b in range(B):
                # AV^T accumulators: [queries-of-iblock, 64 dv] tiles packed
                # as avt[h][:, ib*64:(ib+1)*64], accumulated over all jk.
                avt = []
                for h in range(HL):
                    a = aps.tile([P, JKB * 64], F32, tag=f"avt{h}", bufs=1,
                                 name=f"avt{h}")
                    avt.append(a)
                    # Explicitly zero the banks with full-bank matmuls so the
                    # later 64-col accumulating writes (start=False) always
                    # land on zeroed PSUM regardless of lazy-zero semantics.
                    for bank in range(JKB * 64 // 512):
                        nc.tensor.matmul(
                            a[:, bank * 512 : (bank + 1) * 512],
                            identb, zb, start=True, stop=False,
                            skip_group_check=True,
                        )
                def emit_av(jk, es, nsum):
                    # normalization + AV^T for key-block jk: es blocks are
                    # the stationary lhsT (free dim 128 -> out partitions =
                    # queries), scaled V the 64-wide moving operand.
                    # Accumulates over jk into the pre-zeroed avt banks.
                    vsp = {}
                    for h in range(HL):
                        hs = slice(h * 64, (h + 1) * 64)
                        n1 = ab.tile([P, 1], F32, tag="n1", bufs=4, name="n1")
                        nc.vector.reduce_sum(n1, nsum[h], axis=mybir.AxisListType.X)
                        nr = ab.tile([P, 1], F32, tag="nr", bufs=4, name="nr")
                        nc.vector.reciprocal(nr, n1)
                        vs = ab.tile([P, 64], BF16, tag="vs", bufs=6, name="vs")
                        nc.vector.tensor_scalar_mul(vs, v_sb[b][:, jk, hs], nr)
                        vsp[h] = vs
                    for h in range(HL):
                        for half in range(2):
                            for cb in range(IHALF // 128):
                                ib = half * 8 + cb
                                nc.tensor.matmul(
                                    avt[h][:, ib * 64 : (ib + 1) * 64],
                                    es[(h, half)][:, cb * 128 : (cb + 1) * 128],
                                    vsp[h],
                                    start=False,
                                    stop=(jk == JKB - 1),
                                    skip_group_check=True,
                                )

                # Software pipeline: each jk's AV matmuls are emitted AFTER
                # the next jk's scores, so the in-order PE queue keeps the
                # score->exp chain (the ACT critical path) running ahead.
                pending_av = None
                for jk in range(JKB):
                    # interleave pending W_O chunks (previous batch) between
                    # jk iterations so they hide under this batch's exp time
                    # instead of stalling the ACT pipeline or trailing.
                    es = {}
                    nsum = {}
                    for h in range(HL):
                        nsum[h] = ab.tile([P, 2], F32, tag="nsum", bufs=4, name="nsum")
                    for half in range(2):
                        st = {}
                        for h in range(HL):
                            st[h] = aps.tile(
                                [P, IHALF], F32, tag="st", bufs=2, name="st"
                            )
                        for c in range(NCPH):
                            for h in range(HL):
                                hs = slice(h * 64, (h + 1) * 64)
                                i0 = half * IHALF + c * CS
                                nc.tensor.matmul(
                                    st[h][:, c * CS : (c + 1) * CS],
                                    kT[b][hs, jk * 128 : (jk + 1) * 128],
                                    qT[b][hs, i0 : i0 + CS],
                                    start=True, stop=True,
                                )
                        for h in range(HL):
                            e = ab.tile([P, IHALF], BF16, tag="e", bufs=12, name="e")
                            nc.scalar.activation(
                                e, st[h], AF.Exp,
                                accum_out=nsum[h][:, half : half + 1],
                            )
                            es[(h, half)] = e
                    if pending_av is not None:
                        emit_av(*pending_av)
                    elif wo_pending:
                        emit_wo_chunk(*wo_pending.pop(0))
                    if wo_pending and jk % 2 == 1:
                        emit_wo_chunk(*wo_pending.pop(0))
                    pending_av = (jk, es, nsum)
                emit_av(*pending_av)
                # Drain: avt (PSUM fp32, [q, dv] layout) -> bf16 SBUF ->
                # PE-transpose to AV^T-natural [dv, n] -> strided DVE copies
                # assemble AVc^T chunks ([s*64+dv partitions, r]) for W_O.
                # ps reuses the avt banks (free after the avts copy) so the
                # "st" exp-pipeline slots are never blocked by the drain.
                avts = {}
                for h in range(HL):
                    a_sb = ab.tile([P, JKB * 64], BF16, tag="avts", bufs=2,
                                   name="avts")
                    nc.vector.tensor_copy(a_sb, avt[h])
                    avts[h] = a_sb
                for h in range(HL):
                    ps = aps.tile([P, N], BF16, tag=f"avt{h}", bufs=1, name="ps")
                    for ib in range(JKB):
                        nc.tensor.transpose(
                            ps[0:64, ib * 128 : (ib + 1) * 128],
                            avts[h][:, ib * 64 : (ib + 1) * 64],
                            identb,
                        )
                    act = ab.tile([P, DOUT // P, P], BF16, tag="avct", bufs=4,
                                  name="avct")
                    psr = ps[0:64].rearrange("p (r s2 par) -> p par s2 r",
                                             par=2, s2=8)
                    nc.vector.tensor_copy(act[0:64], psr[:, 0])
                    nc.vector.tensor_copy(act[64:128], psr[:, 1])
                    avct[(b, h)] = act
                wo_pending.extend((b, idx) for idx in range((DOUT // OC) * HL))
            for args in wo_pending:
                emit_wo_chunk(*args)


def build_nc(N=2048, D=1024, DOUT=1024, enable_asserts=False):
    """Build and compile the per-core Bass module. Returns nc."""
    nc = bacc.Bacc(
        "TRN2",
        target_bir_lowering=False,
        debug=False,
        enable_asserts=enable_asserts,
    )
    R = N // 16
    X = nc.dram_tensor("X", [2, N, D], F32, kind="ExternalInput").ap()
    WQ = nc.dram_tensor("WQ", [D, 128], F32, kind="ExternalInput").ap()
    WK = nc.dram_tensor("WK", [D, 128], F32, kind="ExternalInput").ap()
    WV = nc.dram_tensor("WV", [D, 128], F32, kind="ExternalInput").ap()
    WO = nc.dram_tensor("WO", [16 * 64, DOUT], F32, kind="ExternalInput").ap()
    O = nc.dram_tensor("O", [2, 2 * R, DOUT], F32, kind="ExternalOutput").ap()
    with tile.TileContext(nc) as tc:
        build_attn(tc, X, WQ, WK, WV, WO, O, N, D, DOUT)
    nc.compile()
    return nc


_NC_CACHE = {}


def _get_nc():
    if "full" not in _NC_CACHE:
        _NC_CACHE["full"] = build_nc()
    return _NC_CACHE["full"]


class _PjrtRunner:
    """Cached jitted SPMD executor (mirrors bass2jax.run_bass_via_pjrt but
    keeps the jitted callable so repeat calls skip re-trace/re-compile)."""

    def __init__(self, nc, n_cores=8):
        import jax
        from jax.experimental.shard_map import shard_map
        from jax.sharding import Mesh, PartitionSpec
        from concourse import bass2jax

        bass2jax.install_neuronx_cc_hook()
        self.n_cores = n_cores
        partition_name = (
            nc.partition_id_tensor.name if nc.partition_id_tensor else None
        )
        in_names, out_names, out_avals, zero_outs = [], [], [], []
        for alloc in nc.m.functions[0].allocations:
            if not isinstance(alloc, mybir.MemoryLocationSet):
                continue
            name = alloc.memorylocations[0].name
            if alloc.kind == "ExternalInput":
                if name != partition_name:
                    in_names.append(name)
            elif alloc.kind == "ExternalOutput":
                out_names.append(name)
                shape = tuple(alloc.tensor_shape)
                dtype = mybir.dt.np(alloc.dtype)
                out_avals.append(jax.core.ShapedArray(shape, dtype))
                zero_outs.append(np.zeros(shape, dtype))
        self.in_names = in_names
        self.out_names = out_names
        self.out_avals = out_avals
        self.zero_outs = zero_outs
        n_params = len(in_names)
        n_outs = len(out_names)
        all_names = list(in_names + out_names)
        if partition_name is not None:
            all_names.append(partition_name)
        all_names = tuple(all_names)

        def _body(*args):
            operands = list(args)
            if partition_name is not None:
                operands.append(bass2jax.partition_id_tensor())
            outs = bass2jax._bass_exec_p.bind(
                *operands,
                out_avals=tuple(out_avals),
                in_names=all_names,
                out_names=tuple(out_names),
                lowering_input_output_aliases=(),
                sim_require_finite=True,
                sim_require_nnan=True,
                nc=nc,
            )
            return tuple(outs)

        devices = jax.devices()[:n_cores]
        mesh = Mesh(np.asarray(devices), ("core",))
        donate = tuple(range(n_params, n_params + n_outs))
        self._fn = jax.jit(
            shard_map(
                _body,
                mesh=mesh,
                in_specs=(PartitionSpec("core"),) * (n_params + n_outs),
                out_specs=(PartitionSpec("core"),) * n_outs,
                check_rep=False,
            ),
            donate_argnums=donate,
            keep_unused=True,
        )

    def __call__(self, in_maps):
        import jax

        n = self.n_cores
        concat_in = [
            np.concatenate([np.asarray(m[nm]) for m in in_maps], axis=0)
            for nm in self.in_names
        ]
        concat_zeros = [
            np.zeros((n * z.shape[0], *z.shape[1:]), z.dtype) for z in self.zero_outs
        ]
        outs = self._fn(*concat_in, *concat_zeros)
        outs = [np.asarray(o) for o in jax.block_until_ready(outs)]
        return [
            {
                nm: outs[i].reshape(n, *self.out_avals[i].shape)[c]
                for i, nm in enumerate(self.out_names)
            }
            for c in range(n)
        ]


def _get_runner():
    if "runner" not in _NC_CACHE:
        _NC_CACHE["runner"] = _PjrtRunner(_get_nc())
    return _NC_CACHE["runner"]


def _make_in_maps(X, W_Q, W_K, W_V, W_O):
    X = np.ascontiguousarray(np.asarray(X), dtype=np.float32)
    W_Q = np.asarray(W_Q, dtype=np.float32)
    W_K = np.asarray(W_K, dtype=np.float32)
    W_V = np.asarray(W_V, dtype=np.float32)
    W_O = np.ascontiguousarray(np.asarray(W_O), dtype=np.float32)
    in_maps = []
    for c in range(8):
        wq = np.ascontiguousarray(
            np.concatenate([W_Q[2 * c], W_Q[2 * c + 1]], axis=1), dtype=np.float32
        )
        wk = np.ascontiguousarray(
            np.concatenate([W_K[2 * c], W_K[2 * c + 1]], axis=1), dtype=np.float32
        )
        wv = np.ascontiguousarray(
            np.concatenate([W_V[2 * c], W_V[2 * c + 1]], axis=1), dtype=np.float32
        )
        in_maps.append({"X": X, "WQ": wq, "WK": wk, "WV": wv, "WO": W_O})
    return in_maps


def kernel_with_results(X, W_Q, W_K, W_V, W_O, **run_kwargs):
    """Run via run_bass_kernel_spmd (supports trace kwargs); returns results."""
    nc = _get_nc()
    in_maps = _make_in_maps(X, W_Q, W_K, W_V, W_O)
    res = run_bass_kernel_spmd(nc, in_maps, core_ids=list(range(8)), **run_kwargs)
    return np.concatenate([r["O"] for r in res.results], axis=1), res


def kernel(X, W_Q, W_K, W_V, W_O):
    """Full-input entry point. X [2,2048,1024], W_Q/K/V [16,1024,64],
    W_O [1024,1024] -> [2,2048,1024] fp32."""
    try:
        runner = _get_runner()
        results = runner(_make_in_maps(X, W_Q, W_K, W_V, W_O))
        return np.concatenate([r["O"] for r in results], axis=1)
    except Exception:
        out, _ = kernel_with_results(X, W_Q, W_K, W_V, W_O)
        return out
